# revision 2
# baseline (speedup 1.0000x reference)
"""Multi-head self-attention forward on 8 Trainium2 NeuronCores.

Problem: x[4,2048,512] -> qkv proj (w_qkv [512,1536]) -> 8-head attention
(head_dim 64) -> out proj (w_out [512,512] + b_out) -> y[4,2048,512].

Sharding: 8 shards = (batch b in 0..3) x (head-group hg in 0..1, 4 heads each).
Core c handles b=c//2, hg=c%2. Each core computes, for its batch and its 4
heads: qkv projection (only its heads' columns), attention, and the partial
output projection restricted to its heads' rows of w_out. Host sums the two
half-projections per batch and adds the bias.

On-device layout (all "T" tensors keep the contraction dim on partitions):
  xT   [512, 2048]   x[b] transposed (host-side transpose), one SBUF tile
       with the four 128-row chunks side by side in the free dim
  qkT  4 tiles [128, 2048]: Q01, K01, Q23, K23 (2 heads stacked per tile:
       head A on partitions 0:64, head B on 64:128)
  v_aug 16 seq-tiles [128, 4*65]: per head 64 v columns + a ones column
       (the ones column makes the oT matmul also produce the softmax
       denominator as row 64 of its output)
  sT   [k, q] scores transposed -> exp (no max subtraction: |s|~N(0,1), safe
       in fp32) -> pT
  oT   v_aug.T @ pT = [65, q]: rows 0:64 unnormalized head output (d on
       partitions), row 64 = softmax denominator

Out-projection (per block = one head-pair p, one 512-wide q chunk):
  reciprocal of the two denominator rows -> broadcast across 64 partitions
  with a K=33 selector matmul (R2) -> DVE-multiply into oT while casting to
  bf16 (oTs, normalized, heads A/B stacked on partitions) -> single K=128
  matmul per 128-q chunk against w2 covers both heads at once; p=0 result is
  copied to a SBUF accumulator, p=1 is added and DMA'd out. This replaces the
  K=64 matmul pairs + per-head tensor_scalar scaling + gpsimd adds of the
  earlier version (half the yproj PE rows, no transpose matmuls).
"""

import numpy as np

import concourse.bass as bass
import concourse.mybir as mybir
import concourse.tile as tile
from concourse import bacc

DIM = 512
NHEADS = 8
HD = 64
B = 4
SEQ = 2048
SCALE = HD ** -0.5

NCORES = 8
HPC = 4          # heads per core
QCH = 512        # q chunk (moving free dim)
NQC = SEQ // QCH # 4 q-chunks
KCH = 128        # k chunk (psum partition dim)
NKC = SEQ // KCH # 16 k-chunks
CCH = 128        # contraction chunk for projections
NCC = DIM // CCH # 4

F32 = mybir.dt.float32

BF16 = mybir.dt.bfloat16
# matmul input dtype. bf16: 1 cycle/row, FWL weight loads, half the PE power
# of f32r (less HAM throttling). fp8 was evaluated and fails the 2e-2
# correctness gate (rel err ~2.8e-2 in simulation).
MMDT = BF16


def _emit_o(nc, oA, oB, vaug_t, pt_pair, i, p, start, stop):
    """Accumulate the two kc chunks of pair-iteration i into oA/oB."""
    pA, pB = pt_pair
    for hh, (odst, psrc) in enumerate(((oA, pA), (oB, pB))):
        for half in range(2):
            kc = 2 * i + half
            nc.tensor.matmul(
                odst[:],
                vaug_t(kc)[:, 2 * p + hh, :],
                psrc[:, half * QCH:(half + 1) * QCH],
                start=(start and half == 0), stop=(stop and half == 1),
                skip_group_check=True,
            )


def build_nc():
    nc = bacc.Bacc()

    xT_d = nc.dram_tensor("xt", [DIM, SEQ], MMDT, kind="ExternalInput")
    wperm_d = nc.dram_tensor("wperm", [DIM, 4 * 128], MMDT, kind="ExternalInput")
    wv_d = nc.dram_tensor("wv", [DIM, HPC * HD], MMDT, kind="ExternalInput")
    w2_d = nc.dram_tensor("w2", [HPC * HD, DIM], MMDT, kind="ExternalInput")
    y_d = nc.dram_tensor("y", [SEQ, DIM], F32, kind="ExternalOutput")

    with tile.TileContext(nc) as tc:
        with (
            tc.tile_pool(name="const", bufs=1) as cpool,
            tc.tile_pool(name="big", bufs=1) as bigpool,
            tc.tile_pool(name="pt", bufs=4) as ptpool,
            tc.tile_pool(name="yacc", bufs=1) as yaccpool,
            tc.tile_pool(name="tmp", bufs=2) as tmppool,
            tc.tile_pool(name="small", bufs=2) as smallpool,
            tc.tile_pool(name="ps", bufs=1, space="PSUM") as ps,
        ):
            # ---- constants / inputs to SBUF ----
            xTt = cpool.tile([128, NCC * SEQ], MMDT, tag="xT", name="xT")
            wpt = cpool.tile([128, NCC * 512], MMDT, tag="wp", name="wp")
            wvt = cpool.tile([128, NCC * HPC * HD], MMDT, tag="wv", name="wv")
            w2t = cpool.tile([128, 2 * DIM], MMDT, tag="w2", name="w2")
            ones4 = cpool.tile([128, HPC], F32, tag="ones4")
            nc.gpsimd.memset(ones4[:], 1.0)
            ones1 = cpool.tile([1, 1], F32, tag="ones1")
            nc.gpsimd.memset(ones1[:], 1.0)
            # selector for the reciprocal broadcast: row 0 ones, rows 1:33
            # zero (K=33: K=1 matmuls fail an ISA check)
            selst = cpool.tile([33, 64], F32, tag="selst")
            nc.gpsimd.memset(selst[:], 0.0)
            nc.gpsimd.memset(selst[0:1, :], 1.0)
            sel64 = cpool.tile([33, 64], MMDT, tag="sel64")
            nc.vector.tensor_copy(sel64[:], selst[:])
            # preload the exp ACT table set early so the first real exp in
            # the attention phase doesn't stall the pipeline ~2.7us
            dummy = cpool.tile([1, 1], F32, tag="dummy")
            nc.scalar.activation(dummy[:], ones1[:],
                                 mybir.ActivationFunctionType.Exp)

            # ---- input DMA: few big transfers, first-needed first ----
            x4 = xT_d.rearrange("(c p) s -> p c s", p=128)
            xv = xTt.rearrange("p (c s) -> p c s", c=NCC)
            nc.sync.dma_start(
                wpt.rearrange("p (c m) -> p c m", c=NCC),
                wperm_d.rearrange("(c p) m -> p c m", p=128))
            nc.sync.dma_start(xv[:, :, 0:1024], x4[:, :, 0:1024])
            nc.gpsimd.dma_start(
                wvt.rearrange("p (c m) -> p c m", c=NCC),
                wv_d.rearrange("(c p) m -> p c m", p=128))
            nc.gpsimd.dma_start(xv[:, :, 1024:SEQ], x4[:, :, 1024:SEQ])
            nc.gpsimd.dma_start(
                w2t.rearrange("p (g m) -> p g m", g=2),
                w2_d.rearrange("(g p) m -> p g m", p=128))

            def xT_c(c):
                return xTt[:, c * SEQ:(c + 1) * SEQ]

            def wp_c(c):
                return wpt[:, c * 512:(c + 1) * 512]

            def wv_c(c):
                return wvt[:, c * (HPC * HD):(c + 1) * (HPC * HD)]

            def w2_p(p):
                return w2t[:, p * DIM:(p + 1) * DIM]

            # ---- persistent intermediates ----
            qkTs = [bigpool.tile([128, SEQ], MMDT, tag=f"qkT{m}",
                                 name=f"qkT{m}") for m in range(4)]
            vaugs = [bigpool.tile([128, HPC * 65], MMDT, tag=f"vaug{st}",
                                  name=f"vaug{st}") for st in range(NKC)]
            yacc = yaccpool.tile([128, SEQ // 128 * DIM], F32, tag="yacc")

            # zero-init the rcp33 pool bufs once; later writes touch row 0
            # only, so rows 1:33 stay zero for the K=33 broadcast matmul
            for _ in range(2):
                t = smallpool.tile([33, 2 * QCH], MMDT, tag="rcp33")
                nc.gpsimd.memset(t[:], 0.0)

            def qkT_blk(m):
                return qkTs[m]

            def vaug_t(kc):
                return vaugs[kc].rearrange("p (h e) -> p h e", e=65)

            def v_unit(st, tag, bufs):
                pv = ps.tile([128, HPC * HD], F32, tag=tag, bufs=bufs,
                             name="pv")
                for c in range(NCC):
                    nc.tensor.matmul(
                        pv[:],
                        xT_c(c)[:, st * 128:(st + 1) * 128],
                        wv_c(c)[:],
                        start=(c == 0), stop=(c == NCC - 1),
                        skip_group_check=True,
                    )
                vt = vaug_t(st)
                nc.vector.tensor_copy(
                    vt[:, :, 0:64], pv[:].rearrange("p (h d) -> p h d", d=HD)
                )
                nc.vector.tensor_copy(
                    vt[:, :, 64:65],
                    ones4[:].rearrange("p (h o) -> p h o", o=1))

            def qk_unit(m, s2, tag, bufs):
                pp = ps.tile([128, 512], F32, tag=tag, bufs=bufs, name="pp")
                for c in range(NCC):
                    nc.tensor.matmul(
                        pp[:],
                        wp_c(c)[:, m * 128:(m + 1) * 128],
                        xT_c(c)[:, s2 * 512:(s2 + 1) * 512],
                        start=(c == 0), stop=(c == NCC - 1),
                        skip_group_check=True,
                    )
                nc.vector.tensor_copy(qkTs[m][:, s2 * 512:(s2 + 1) * 512],
                                      pp[:])

            # ---- phase 1a: qkT = wperm.T @ xT (Q01/K01 now; Q23/K23 are
            # filler inside the first attention block) ----
            for m in range(2):
                for s in range(SEQ // 1024):
                    pp = ps.tile([128, 1024], F32, tag="sA", bufs=1, name="pp")
                    for c in range(NCC):
                        for half in range(2):
                            nc.tensor.matmul(
                                pp[:, half * 512:(half + 1) * 512],
                                wp_c(c)[:, m * 128:(m + 1) * 128],
                                xT_c(c)[:, s * 1024 + half * 512:
                                        s * 1024 + (half + 1) * 512],
                                start=(c == 0),
                                stop=(c == NCC - 1),
                                skip_group_check=True,
                            )
                    nc.vector.tensor_copy(
                        qkT_blk(m)[:, s * 1024:(s + 1) * 1024], pp[:]
                    )

            # ---- phase 1b: first v seq-tiles (rest are attention filler) --
            for st in range(4):
                v_unit(st, "sB", 1)

            filler = [lambda st=st: v_unit(st, "y", 2) for st in range(4, NKC)]
            filler += [lambda m=m, s2=s2: qk_unit(m, s2, "y", 2)
                       for m in (2, 3) for s2 in range(4)]

            # ---- out-projection helpers ----
            def emit_norm(p, qc, oA, oB):
                """Block epilogue: normalize oA/oB into a bf16 [128, QCH]
                tile (heads stacked on partitions) using the denominator
                rows. Runs right after the block so oA/oB free early."""
                rcps = smallpool.tile([1, 2 * QCH], F32, tag="rcps")
                nc.vector.reciprocal(rcps[:, 0:QCH], oA[64:65, :])
                nc.vector.reciprocal(rcps[:, QCH:2 * QCH], oB[64:65, :])
                rcp33 = smallpool.tile([33, 2 * QCH], MMDT, tag="rcp33")
                nc.vector.tensor_copy(rcp33[0:1, :], rcps[:])
                R2a = ps.tile([64, QCH], F32, tag="y", bufs=2, name="R2a")
                nc.tensor.matmul(R2a[:], sel64[:], rcp33[:, 0:QCH],
                                 start=True, stop=True, skip_group_check=True)
                R2b = ps.tile([64, QCH], F32, tag="y", bufs=2, name="R2b")
                nc.tensor.matmul(R2b[:], sel64[:], rcp33[:, QCH:2 * QCH],
                                 start=True, stop=True, skip_group_check=True)
                R2sb = smallpool.tile([64, 2 * QCH], F32, tag="R2sb")
                nc.vector.tensor_copy(R2sb[:, 0:QCH], R2a[:])
                nc.vector.tensor_copy(R2sb[:, QCH:2 * QCH], R2b[:])
                ot = tmppool.tile([128, QCH], MMDT, tag="oTs")
                nc.vector.tensor_mul(ot[0:64, :], oA[0:64, :],
                                     R2sb[:, 0:QCH])
                nc.vector.tensor_mul(ot[64:128, :], oB[0:64, :],
                                     R2sb[:, QCH:2 * QCH])
                return {"p": p, "qc": qc, "ot": ot}

            def emit_yproj_j(pend, j):
                p, qc, ot = pend["p"], pend["qc"], pend["ot"]
                qt = qc * (QCH // 128) + j
                yps = ps.tile([128, DIM], F32, tag="y", bufs=2, name="yps")
                nc.tensor.matmul(
                    yps[:],
                    ot[:, j * 128:(j + 1) * 128],
                    w2_p(p)[:],
                    start=True, stop=True, skip_group_check=True,
                )
                ya = yacc[:, qt * DIM:(qt + 1) * DIM]
                if p == 0:
                    nc.vector.tensor_copy(ya, yps[:])
                else:
                    nc.vector.tensor_add(ya, ya, yps[:])
                    nc.sync.dma_start(y_d[qt * 128:(qt + 1) * 128, :], ya)

            # ---- phase 2: attention + out-proj ----
            # kc chunks processed in pairs: one s psum tile [128, 1024] holds
            # scores for kc and kc+1 side by side, halving ACT instruction
            # count. Two levels of software pipelining keep the PE stream
            # dense: within a block, s(i+1) is emitted before o(i) so the PE
            # never head-of-line blocks on exp(i); across blocks, the
            # out-projection of block n is spread into the first
            # pair-iterations of block n+1.
            NPAIR = NKC // 2

            pending = None
            for p in range(2):
                Q = qkT_blk(2 * p)
                K = qkT_blk(2 * p + 1)
                for qc in range(NQC):
                    oA = ps.tile([65, QCH], F32, tag="oA", bufs=1, name="oA")
                    oB = ps.tile([65, QCH], F32, tag="oB", bufs=1, name="oB")
                    prev = None
                    for i in range(NPAIR):
                        sA = ps.tile([128, 2 * QCH], F32, tag="sA", bufs=1,
                                     name="sA")
                        sB = ps.tile([128, 2 * QCH], F32, tag="sB", bufs=1,
                                     name="sB")
                        for hh, stile in ((0, sA), (1, sB)):
                            for half in range(2):
                                kc = 2 * i + half
                                nc.tensor.matmul(
                                    stile[:, half * QCH:(half + 1) * QCH],
                                    K[64 * hh:64 * hh + 64,
                                      kc * 128:(kc + 1) * 128],
                                    Q[64 * hh:64 * hh + 64,
                                      qc * QCH:(qc + 1) * QCH],
                                    start=True, stop=True,
                                    skip_group_check=True,
                                )
                        if filler and p == 0:
                            filler.pop(0)()
                            if filler and qc == 0 and i >= 4:
                                filler.pop(0)()
                        # previous block's out-projection, spread across
                        # this block's early pair-iterations
                        if pending is not None and 2 <= i < 2 + QCH // 128:
                            emit_yproj_j(pending, i - 2)
                            if i == 1 + QCH // 128:
                                pending = None
                        if prev is not None:
                            _emit_o(nc, oA, oB, vaug_t, prev, i - 1, p,
                                    start=(i == 1), stop=(i == NPAIR - 1))
                        pA = ptpool.tile([128, 2 * QCH], MMDT, tag="pA")
                        pB = ptpool.tile([128, 2 * QCH], MMDT, tag="pB")
                        nc.scalar.activation(
                            pA[:], sA[:], mybir.ActivationFunctionType.Exp,
                            scale=SCALE,
                        )
                        nc.scalar.activation(
                            pB[:], sB[:], mybir.ActivationFunctionType.Exp,
                            scale=SCALE,
                        )
                        prev = (pA, pB)
                    _emit_o(nc, oA, oB, vaug_t, prev, NPAIR - 1, p,
                            start=(NPAIR == 1), stop=True)
                    pending = emit_norm(p, qc, oA, oB)

            # tail: last block's out-projection
            for j in range(QCH // 128):
                emit_yproj_j(pending, j)

    nc.finalize()
    return nc


_NC_CACHE = {}


def get_nc():
    if "nc" not in _NC_CACHE:
        _NC_CACHE["nc"] = build_nc()
    return _NC_CACHE["nc"]


def make_core_inputs(x, w_qkv, w_out):
    """Per-core input dicts (host-side sharding)."""
    in_maps = []
    for c in range(NCORES):
        b, hg = c // 2, c % 2
        heads = [hg * HPC + i for i in range(HPC)]
        qcols = [w_qkv[:, h * HD:(h + 1) * HD] for h in heads]
        kcols = [w_qkv[:, DIM + h * HD:DIM + (h + 1) * HD] for h in heads]
        vcols = [w_qkv[:, 2 * DIM + h * HD:2 * DIM + (h + 1) * HD] for h in heads]
        wperm = np.concatenate(
            [qcols[0], qcols[1], kcols[0], kcols[1],
             qcols[2], qcols[3], kcols[2], kcols[3]], axis=1)
        wv = np.concatenate(vcols, axis=1)
        w2 = w_out[hg * HPC * HD:(hg + 1) * HPC * HD, :]
        import ml_dtypes
        mmnp = (ml_dtypes.bfloat16 if MMDT == mybir.dt.bfloat16
                else np.float32)
        in_maps.append({
            "xt": np.ascontiguousarray(x[b].T).astype(mmnp),
            "wperm": np.ascontiguousarray(wperm).astype(mmnp),
            "wv": np.ascontiguousarray(wv).astype(mmnp),
            "w2": np.ascontiguousarray(w2).astype(mmnp),
        })
    return in_maps


def kernel(x, w_qkv, w_out, b_out):
    from concourse.bass_utils import run_bass_kernel_spmd

    x = np.asarray(x, dtype=np.float32)
    w_qkv = np.asarray(w_qkv, dtype=np.float32)
    w_out = np.asarray(w_out, dtype=np.float32)
    b_out = np.asarray(b_out, dtype=np.float32)

    nc = get_nc()
    in_maps = make_core_inputs(x, w_qkv, w_out)
    res = run_bass_kernel_spmd(nc, in_maps, list(range(NCORES))).results

    out = np.empty((B, SEQ, DIM), dtype=np.float32)
    for b in range(B):
        out[b] = res[2 * b]["y"] + res[2 * b + 1]["y"] + b_out
    return out
